# revision 13
# baseline (speedup 1.0000x reference)
"""BevFeatureEncoder on 8 Trainium2 NeuronCores.

Strategy (data-parallel over BEV grid slabs):
  - The 2*480*360 BEV cells are split into 8 contiguous ranges of 43200
    cells; points are routed on host to the core owning their cell, so
    the segment_max reduction is fully local to each core.
  - Per core, occupied cells are sorted by point count DESC and grouped
    into chunks of 2048 cells. Slot s of a chunk covers only the prefix
    of cells with count > s (widths shrink with s, rounded to 256), so
    there is no power-of-2 slot padding. Items run in s-major order so
    the per-chunk max-accumulate chains never serialize back-to-back.
  - Device dataflow: 3-layer MLP as bf16 matmuls with BN scale folded
    into the weights; each BN+ReLU is one wide (N=2048) PSUM drain on
    ACT or DVE, debt-balanced. Layer 1 packs two cells per column
    (block-diag [8,128] stationary) and is 4-way row-tiled via base
    partitions {0,32,64,96}; layer 2 is 2-way row-tiled (K=64 halves at
    partitions 0/64); the compression runs col-tiled (out partitions
    0:64 / 64:128 for a chunk pair). Max accumulation is an in-place
    scalar_tensor_tensor on DVE over bf16 SBUF accumulator planes.
  - Output is compacted [64, G*2048] bf16; host places real columns
    into the zeroed [B, C, GX, GY] grid. Chunk structure is equalized
    across cores (max widths) so one SPMD program serves all 8 cores.
"""

import numpy as np

import concourse.bacc as bacc
import concourse.bass as bass
import concourse.mybir as mybir
import concourse.tile as tile
from concourse import bass_utils
from concourse.masks import make_identity

GX, GY = 480, 360
B = 2
EPS = 1e-5
N_CORES = 8
CELLS_PER_CORE = (B * GX * GY) // N_CORES  # 43200
CHUNK = 2048  # cells per chunk (fin granularity; 4 PSUM banks wide)
PAD = -1

F32 = mybir.dt.float32
BF16 = mybir.dt.bfloat16

Relu = mybir.ActivationFunctionType.Relu


# ---------------------------------------------------------------- host prep


def _build_plan_and_data(voxels, coors):
    """Route points to cores; build the equalized slot plan and per-core
    device inputs.

    Returns:
      plan: (G, widths) where widths[s][g] = item width (0 = absent),
      item list implicitly s-major over g.
      vox_all: [N_CORES, 104, G1*512] packed voxel features (bf16 later).
      rows_all: [N_CORES, G*2048] local slab row per padded cell (PAD =
      dummy).
      h1map: list of (s, g, w, grp, off) per item in emission order.
      G1: number of 2048-col h1 groups.
    """
    seg = (
        coors[:, 0].astype(np.int64) * (GX * GY)
        + coors[:, 1].astype(np.int64) * GY
        + coors[:, 2].astype(np.int64)
    )
    core_of = seg // CELLS_PER_CORE

    per_core = []  # (cells_sorted, counts_sorted, slot_pts[list of arrays])
    for c in range(N_CORES):
        idx = np.nonzero(core_of == c)[0]
        seg_local = seg[idx] - c * CELLS_PER_CORE
        order = np.argsort(seg_local, kind="stable")
        seg_sorted = seg_local[order]
        cells, starts, counts = np.unique(
            seg_sorted, return_index=True, return_counts=True
        )
        # sort cells by count desc (stable on cell id)
        o2 = np.argsort(-counts, kind="stable")
        cells, starts, counts = cells[o2], starts[o2], counts[o2]
        pts = idx[order]  # global point idx in seg-sorted order
        per_core.append((cells, starts, counts, pts))

    n_occ_max = max(len(pc[0]) for pc in per_core)
    G = -(-n_occ_max // CHUNK)
    if G % 2:
        G += 1  # fin works on chunk pairs
    kmax = int(max(pc[2].max() for pc in per_core))

    # widths[s][g]: cells with count > s in chunk g, maxed over cores,
    # rounded up to 256 (0 if no core has any)
    widths = np.zeros((kmax, G), np.int64)
    for cells, starts, counts, pts in per_core:
        n = len(cells)
        for g in range(G):
            lo, hi = g * CHUNK, min((g + 1) * CHUNK, n)
            if lo >= hi:
                continue
            cg = counts[lo:hi]
            for s in range(int(cg[0])):  # counts desc within chunk
                widths[s, g] = max(widths[s, g], int((cg > s).sum()))
    widths = np.minimum(-(-widths // 256) * 256, CHUNK)

    # items in s-major order; greedy-pack their h1 blocks (w//2 cols)
    # into 2048-col groups so every item's h1 is contiguous in one tile
    items = []  # (s, g, w, grp, off)
    grp, off = 0, 0
    for s in range(kmax):
        for g in range(G):
            w = int(widths[s, g])
            if w == 0:
                continue
            h = w // 2
            if off + h > 2048:
                grp += 1
                off = 0
            items.append((s, g, w, grp, off))
            off += h
    G1 = grp + 1

    # vox layout: [8, G1*2048] flat packed columns
    vox_all = np.zeros((N_CORES, 8, G1 * 2048), np.float32)
    rows_all = np.full((N_CORES, G * CHUNK), PAD, np.int64)

    for core, (cells, starts, counts, pts) in enumerate(per_core):
        n = len(cells)
        rows_all[core, :n] = cells
        for (s, g, w, gp, of) in items:
            lo = g * CHUNK
            ncell = max(0, min(w, n - lo))
            if ncell == 0:
                continue  # vox stays zero
            # point index per cell for slot s (cells lo..lo+ncell-1):
            # point s if count > s else point 0 (dup is a no-op under max)
            cnt = counts[lo : lo + ncell]
            so = np.minimum(s, cnt - 1)
            p_idx = pts[starts[lo : lo + ncell] + so]
            feats = np.zeros((w, 4), np.float32)
            feats[:ncell] = voxels[p_idx]
            # pack two cells per column: col j = cell j (rows r0:r0+4)
            # over cell w//2+j (rows r0+4:r0+8)
            h = w // 2
            packed = np.concatenate([feats[:h].T, feats[h:].T], axis=0)  # [8,h]
            pc0 = gp * 2048 + of
            vox_all[core, :, pc0 : pc0 + h] = packed
    return (G, kmax, tuple(widths.flatten().tolist())), items, G1, \
        vox_all, rows_all


# ------------------------------------------------------------- bass program


def build_program(G, items, G1):
    nc = bacc.Bacc("TRN2", target_bir_lowering=False, debug=False,
                   num_devices=N_CORES)

    vox = nc.dram_tensor("vox", [8, G1 * 2048], BF16,
                         kind="ExternalInput").ap()
    w_in = {}
    for name, shape in [
        ("w1", [4, 64]), ("w2", [64, 128]), ("w3", [128, 256]),
        ("wc", [256, 64]), ("bc", [64]),
        ("g1", [64]), ("b1", [64]), ("m1", [64]), ("v1", [64]),
        ("g2", [128]), ("b2", [128]), ("m2", [128]), ("v2", [128]),
        ("g3", [256]), ("b3", [256]), ("m3", [256]), ("v3", [256]),
    ]:
        w_in[name] = nc.dram_tensor(name, shape, F32, kind="ExternalInput").ap()
    comp = nc.dram_tensor("comp", [64, G * CHUNK], BF16,
                          kind="ExternalOutput").ap()

    from contextlib import ExitStack
    with tile.TileContext(nc) as tc, ExitStack() as ctx:
        cpool = ctx.enter_context(tc.tile_pool(name="const", bufs=1))

        ident = cpool.tile([128, 128], F32)
        make_identity(nc, ident[:])
        eps_tile = cpool.tile([128, 1], F32)
        nc.vector.memset(eps_tile[:], EPS)

        # vox preload: two DMAs on idle queues
        vox_sb = cpool.tile([8, G1 * 2048], BF16)
        half = (G1 // 2) * 2048
        if half:
            nc.sync.dma_start(out=vox_sb[:, 0:half], in_=vox[:, 0:half])
        nc.gpsimd.dma_start(out=vox_sb[:, half:], in_=vox[:, half:])

        _bnq = [nc.gpsimd, nc.scalar, nc.scalar, nc.gpsimd]

        with tc.tile_pool(name="fold_ps", bufs=2, space="PSUM") as fps, \
             tc.tile_pool(name="fold_sb", bufs=2) as fsb:

            def bn_scale_bias(li, c, half=None):
                sfx = f"{li}_{half}"
                rows = fsb.tile([4, c], F32, tag="bn_rows")
                for i, pfx in enumerate("gbmv"):
                    src = w_in[f"{pfx}{li}"]
                    if half is not None:
                        src = src[half * 128 : half * 128 + c]
                    _bnq[i].dma_start(out=rows[i : i + 1, :], in_=src[None, :])
                pT = fps.tile([c, 4], F32, tag="bn_pT", space="PSUM")
                nc.tensor.transpose(out=pT[:], in_=rows[:],
                                    identity=ident[:4, :4])
                cols = cpool.tile([c, 4], F32, tag=f"bn_cols{sfx}")
                nc.vector.tensor_copy(cols[:], pT[:])
                g, b, m, v = (cols[:, i : i + 1] for i in range(4))
                sq = cpool.tile([c, 1], F32, tag=f"bn_sq{sfx}")
                nc.scalar.activation(sq[:], v,
                                     mybir.ActivationFunctionType.Sqrt,
                                     bias=eps_tile[:c, :])
                s = cpool.tile([c, 1], F32, tag=f"bn_s{sfx}")
                nc.vector.reciprocal(s[:], sq[:])
                nc.vector.tensor_mul(s[:], g, s[:])
                t = cpool.tile([c, 1], F32, tag=f"bn_t{sfx}")
                nc.vector.tensor_mul(t[:], m, s[:])
                nc.vector.tensor_sub(t[:], b, t[:])
                return s, t

            def fold(name, kin, cout, w_src, s_ap):
                wr = fsb.tile([kin, cout], F32, tag="fold_raw")
                nc.gpsimd.dma_start(out=wr[:], in_=w_src)
                pT = fps.tile([cout, kin], F32, tag="fold_pT", space="PSUM")
                nc.tensor.transpose(out=pT[:], in_=wr[:],
                                    identity=ident[:kin, :kin])
                wT = fsb.tile([cout, kin], F32, tag="fold_wT")
                nc.vector.tensor_copy(wT[:], pT[:])
                dg = fsb.tile([cout, cout], F32, tag="fold_dg")
                nc.vector.tensor_scalar_mul(dg[:], ident[:cout, :cout], s_ap)
                pS = fps.tile([cout, kin], F32, tag="fold_pS", space="PSUM")
                nc.tensor.matmul(pS[:], dg[:], wT[:], start=True, stop=True)
                wsT = fsb.tile([cout, kin], F32, tag="fold_wsT")
                nc.vector.tensor_copy(wsT[:], pS[:])
                pB = fps.tile([kin, cout], F32, tag="fold_pB", space="PSUM")
                nc.tensor.transpose(out=pB[:], in_=wsT[:],
                                    identity=ident[:cout, :cout])
                out = cpool.tile([kin, cout], BF16, tag=name)
                nc.vector.tensor_copy(out[:], pB[:])
                return out

            # layer 1: block-diag [8,128] replicated at partitions
            # {0,32,64,96} for 4-way row tiling
            s1, t1 = bn_scale_bias(1, 64)
            w1s4 = fold("w1s4", 4, 64, w_in["w1"], s1[:])
            w1r = cpool.tile([8, 128], BF16)
            nc.vector.memset(w1r[:], 0.0)
            nc.vector.tensor_copy(w1r[0:4, 0:64], w1s4[:])
            nc.gpsimd.dma_start(out=w1r[4:8, 64:128], in_=w1s4[:])
            t1d2 = cpool.tile([128, 1], F32)
            nc.vector.tensor_copy(t1d2[0:64, :], t1[:])
            nc.gpsimd.dma_start(out=t1d2[64:128, :], in_=t1[:])

            # layer 2: [64,128] stationary replicated at partitions 0/64
            # for 2-way row tiling
            s2, t2 = bn_scale_bias(2, 128)
            w2s = fold("w2s", 64, 128, w_in["w2"], s2[:])
            MM2_TILED = False
            if MM2_TILED:
                w2d = cpool.tile([128, 128], BF16)
                nc.vector.tensor_copy(w2d[0:64, :], w2s[:])
                nc.gpsimd.dma_start(out=w2d[64:128, :], in_=w2s[:])
            else:
                w2e = cpool.tile([128, 128], BF16)
                nc.vector.memset(w2e[:], 0.0)
                nc.vector.tensor_copy(w2e[0:64, :], w2s[:])
                w2o = cpool.tile([128, 128], BF16)
                nc.vector.memset(w2o[:], 0.0)
                nc.gpsimd.dma_start(out=w2o[64:128, :], in_=w2s[:])

            # layer 3 halves
            s3a, t3a = bn_scale_bias(3, 128, half=0)
            w3a = fold("w3a", 128, 128, w_in["w3"][:, 0:128], s3a[:])
            s3b, t3b = bn_scale_bias(3, 128, half=1)
            w3b = fold("w3b", 128, 128, w_in["w3"][:, 128:256], s3b[:])

        def load_cast(name, shape, src_ap):
            raw = cpool.tile(shape, F32, tag=name + "_raw")
            nc.gpsimd.dma_start(out=raw[:], in_=src_ap)
            rnd = cpool.tile(shape, BF16, tag=name)
            nc.vector.tensor_copy(rnd[:], raw[:])
            return rnd

        wc0 = load_cast("wc0", [128, 64], w_in["wc"][0:128, :])
        wc1 = load_cast("wc1", [128, 64], w_in["wc"][128:256, :])
        bc2 = cpool.tile([128, 1], F32)
        nc.scalar.dma_start(out=bc2[0:64, :], in_=w_in["bc"][:, None])
        nc.scalar.dma_start(out=bc2[64:128, :], in_=w_in["bc"][:, None])

        # accumulator planes (bf16, in-place max) — garbage cols beyond
        # written widths are dropped at unshard
        accA = cpool.tile([128, G * CHUNK], BF16)
        accB = cpool.tile([128, G * CHUNK], BF16)
        # zero acc cols never written by any slot (pad cells): fin reads
        # them, host drops the result
        w0 = {}
        for (s, g, w, gp, of) in items:
            w0[g] = max(w0.get(g, 0), w)
        for g in range(G):
            wg = w0.get(g, 0)
            if wg < CHUNK:
                nc.vector.memset(accA[:, g * CHUNK + wg : (g + 1) * CHUNK], 0.0)
                nc.vector.memset(accB[:, g * CHUNK + wg : (g + 1) * CHUNK], 0.0)

        h1p = ctx.enter_context(tc.tile_pool(name="h1p", bufs=4))
        h2p = ctx.enter_context(tc.tile_pool(name="h2p", bufs=5))
        scp = ctx.enter_context(tc.tile_pool(name="scp", bufs=2))
        psum = ctx.enter_context(tc.tile_pool(name="psum", bufs=2,
                                              space="PSUM"))

        # drain routing: debt-balanced over ACT / DVE
        debt = {"act": 0.0, "dve": 0.0}
        COST = {
            "act": lambda n: (n + 352.0) / 1.2,
            "dve": lambda n: (n + 120.0) / 0.96,
        }

        def br_auto(out_ap, in_ap, bias_ap):
            n = in_ap.shape[-1]
            eng = min(("act", "dve"), key=lambda e: debt[e] + COST[e](n))
            debt[eng] += COST[eng](n)
            if eng == "act":
                nc.scalar.activation(out_ap, in_ap, Relu, bias=bias_ap,
                                     scale=1.0)
            else:
                nc.vector.tensor_scalar(out_ap, in_ap, bias_ap, 0.0,
                                        op0=mybir.AluOpType.add,
                                        op1=mybir.AluOpType.max)

        def max_br(out_ap, in_ap, bias_ap, acc_ap):
            debt["dve"] += COST["dve"](in_ap.shape[-1])
            nc.vector.scalar_tensor_tensor(out_ap, in_ap, bias_ap, acc_ap,
                                           op0=mybir.AluOpType.add,
                                           op1=mybir.AluOpType.max)

        h1t = {}  # grp -> tile

        def unit_p1(t):
            """mm1 for h1 group t: 4 matmuls + one drain."""
            p = psum.tile([128, 2048], F32, tag="ps", space="PSUM",
                          name=f"p1_{t}")
            slab = vox_sb[:, t * 2048 : (t + 1) * 2048]
            for i in range(4):
                nc.tensor.matmul(p[:, i * 512 : (i + 1) * 512],
                                 w1r[:],
                                 slab[:, i * 512 : (i + 1) * 512],
                                 start=True, stop=True)
            h1 = h1p.tile([128, 2048], BF16, tag="h1", name=f"h1_{t}")
            br_auto(h1[:], p[:], t1d2[:])
            h1t[t] = h1

        def unit_p2(it):
            (s, g, w, gp, of) = it
            h = w // 2
            h1s = h1t[gp]
            p = psum.tile([128, 2048], F32, tag="ps", space="PSUM",
                          name=f"p2_{s}_{g}")[:, 0:w]
            def seg(c0, width):
                # split [c0, c0+width) at 512-col PSUM bank boundaries
                j = c0
                while j < c0 + width:
                    j1 = min((j // 512 + 1) * 512, c0 + width)
                    yield j, j1
                    j = j1

            if MM2_TILED:
                for j, j1 in seg(0, h):
                    nc.tensor.matmul(p[:, j:j1], w2d[0:64, :],
                                     h1s[0:64, of + j : of + j1],
                                     start=True, stop=True)
                for j, j1 in seg(h, h):
                    nc.tensor.matmul(p[:, j:j1], w2d[64:128, :],
                                     h1s[64:128, of + j - h : of + j1 - h],
                                     start=True, stop=True)
            else:
                for j, j1 in seg(0, h):
                    nc.tensor.matmul(p[:, j:j1], w2e[:],
                                     h1s[:, of + j : of + j1],
                                     start=True, stop=True)
                for j, j1 in seg(h, h):
                    nc.tensor.matmul(p[:, j:j1], w2o[:],
                                     h1s[:, of + j - h : of + j1 - h],
                                     start=True, stop=True)
            h2 = h2p.tile([128, 2048], BF16, tag="h2",
                          name=f"h2_{s}_{g}")[:, 0:w]
            br_auto(h2, p, t2[:])
            return h2

        def unit_p3(it, h2_ap, half):
            (s, g, w, gp, of) = it
            wst = w3a if half == 0 else w3b
            bias = t3a if half == 0 else t3b
            acc = (accA if half == 0 else accB)[:, g * CHUNK : g * CHUNK + w]
            p = psum.tile([128, 2048], F32, tag="ps", space="PSUM",
                          name=f"p3{half}_{s}_{g}")[:, 0:w]
            for j in range(0, w, 512):
                j1 = min(j + 512, w)
                nc.tensor.matmul(p[:, j:j1], wst[:], h2_ap[:, j:j1],
                                 start=True, stop=True)
            if s == 0:
                br_auto(acc, p, bias[:])
            else:
                max_br(acc, p, bias[:], acc)

        def unit_fin(g):
            """compression for chunk pair (g, g+1), col-tiled via out
            partitions 0:64 / 64:128."""
            p = psum.tile([128, 2048], F32, tag="ps", space="PSUM",
                          name=f"pc_{g}")
            for q in range(2):
                cols = (g + q) * CHUNK
                dst = p[64 * q : 64 * q + 64, :]
                for j in range(0, 2048, 512):
                    nc.tensor.matmul(dst[:, j : j + 512], wc0[:],
                                     accA[:, cols + j : cols + j + 512],
                                     start=True, stop=False)
                    nc.tensor.matmul(dst[:, j : j + 512], wc1[:],
                                     accB[:, cols + j : cols + j + 512],
                                     start=False, stop=True)
            sc = scp.tile([128, 2048], BF16, tag="sc", name=f"sc_{g}")
            br_auto(sc[:], p[:], bc2[:])
            o = g * CHUNK
            nc.gpsimd.dma_start(out=comp[:, o : o + CHUNK], in_=sc[0:64, :])
            nc.gpsimd.dma_start(out=comp[:, o + CHUNK : o + 2 * CHUNK],
                                in_=sc[64:128, :])

        # ---- software-pipelined emission ----
        # chunk completion tracking for fins
        last_s = {}
        for (s, g, w, gp, of) in items:
            last_s[g] = s
        done = [False] * G
        finned = [False] * G

        def try_fins(g):
            done[g] = True
            gp = g - (g % 2)
            if done[gp] and done[gp + 1] and not finned[gp]:
                finned[gp] = True
                unit_fin(gp)

        # dummy chunks (no items at all) are done from the start
        have_items = {g for (_, g, _, _, _) in items}
        for g in range(G):
            if g not in have_items:
                done[g] = True
        for g in range(0, G, 2):
            if done[g] and done[g + 1]:
                finned[g] = True  # acc garbage; skip compute, host drops

        p2q = list(items)  # items awaiting phase 2, in order
        p3q = []  # (item, h2_ap) awaiting phase 3
        next_t = 0

        def pump(grp_done_upto):
            """emit ready phase-2/3 units given h1 groups < grp_done_upto."""
            while p3q:
                it, h2a = p3q.pop(0)
                unit_p3(it, h2a, 0)
                unit_p3(it, h2a, 1)
                if it[0] == last_s[it[1]]:
                    try_fins(it[1])
            while p2q and p2q[0][3] < grp_done_upto:
                it = p2q.pop(0)
                p3q.append((it, unit_p2(it)))

        for t in range(G1):
            unit_p1(t)
            next_t = t + 1
            pump(next_t - 1)
        pump(G1)
        pump(G1)

    nc.compile()
    return nc


# ------------------------------------------------------------------ driver

_CACHE = {}


def kernel(voxels, coors, batch_size, w1, g1, b1, m1, v1,
           w2, g2, b2, m2, v2, w3, g3, b3, m3, v3, wc, bc,
           _trace=False):
    voxels = np.asarray(voxels, np.float32)
    coors = np.asarray(coors, np.int32)
    plan_key, items, G1, vox_all, rows_all = _build_plan_and_data(
        voxels, coors)
    G = plan_key[0]

    if plan_key not in _CACHE:
        _CACHE[plan_key] = build_program(G, items, G1)
    nc = _CACHE[plan_key]

    weights = {
        k: np.asarray(v, np.float32)
        for k, v in [
            ("w1", w1), ("w2", w2), ("w3", w3), ("wc", wc), ("bc", bc),
            ("g1", g1), ("b1", b1), ("m1", m1), ("v1", v1),
            ("g2", g2), ("b2", b2), ("m2", m2), ("v2", v2),
            ("g3", g3), ("b3", b3), ("m3", m3), ("v3", v3),
        ]
    }
    import ml_dtypes
    in_maps = [
        {"vox": vox_all[c].astype(ml_dtypes.bfloat16), **weights}
        for c in range(N_CORES)
    ]
    res = bass_utils.run_bass_kernel_spmd(
        nc, in_maps, core_ids=list(range(N_CORES)), trace=_trace)

    out = np.zeros((B, 64, GX * GY), np.float32)
    for c in range(N_CORES):
        cm = np.asarray(res.results[c]["comp"]).astype(np.float32)
        rows = rows_all[c]
        real = rows != PAD
        gcell = rows[real] + c * CELLS_PER_CORE
        b_core = c // (N_CORES // B)
        xy = gcell - b_core * (GX * GY)
        out[b_core][:, xy] = cm[:, real]
    out = out.reshape(B, 64, GX, GY)
    if _trace:
        return out, res
    return out


# revision 16
# speedup vs baseline: 1.0661x; 1.0661x over previous
"""BevFeatureEncoder on 8 Trainium2 NeuronCores.

Strategy (data-parallel over BEV grid slabs):
  - The 2*480*360 BEV cells are split into 8 contiguous ranges of 43200
    cells; points are routed on host to the core owning their cell, so
    the segment_max reduction is fully local to each core.
  - Per core, occupied cells are sorted by point count DESC and grouped
    into chunks of 2048 cells. Slot s of a chunk covers only the prefix
    of cells with count > s (widths shrink with s, rounded to 256), so
    there is no power-of-2 slot padding. Items run in s-major order so
    the per-chunk max-accumulate chains never serialize back-to-back.
  - BN scale/shift is folded into the weights ON HOST (numpy): the
    device sees pre-folded bf16 stationaries + per-partition biases and
    has no weight-prep preamble.
  - Device dataflow: 2048-wide units, each 4 matmuls into a 4-bank PSUM
    tile + ONE wide drain (relu+bias) on ACT or DVE, debt-balanced.
    Layer 1 packs two cells per column (block-diag [8,128]); layer 2
    unpacks via even/odd zero-padded stationaries; the compression runs
    col-tiled (out partitions 0:64 / 64:128 for a chunk pair). Max
    accumulation (slots s>0) is split: ACT/DVE relu-drain to a temp
    tile, then a 2x-rate bf16 SBUF tensor_tensor max into the
    accumulator planes.
  - Output is compacted [64, G*2048] bf16; host places real columns
    into the zeroed [B, C, GX, GY] grid. Chunk structure is equalized
    across cores (max widths) so one SPMD program serves all 8 cores.
"""

import numpy as np

import concourse.bacc as bacc
import concourse.bass as bass
import concourse.mybir as mybir
import concourse.tile as tile
from concourse import bass_utils

GX, GY = 480, 360
B = 2
EPS = 1e-5
N_CORES = 8
CELLS_PER_CORE = (B * GX * GY) // N_CORES  # 43200
CHUNK = 2048  # cells per chunk (fin granularity; 4 PSUM banks wide)
PAD = -1

F32 = mybir.dt.float32
BF16 = mybir.dt.bfloat16

Relu = mybir.ActivationFunctionType.Relu


# ---------------------------------------------------------------- host prep


def _build_plan_and_data(voxels, coors):
    """Route points to cores; build the equalized slot plan and per-core
    device inputs."""
    seg = (
        coors[:, 0].astype(np.int64) * (GX * GY)
        + coors[:, 1].astype(np.int64) * GY
        + coors[:, 2].astype(np.int64)
    )
    core_of = seg // CELLS_PER_CORE

    per_core = []
    for c in range(N_CORES):
        idx = np.nonzero(core_of == c)[0]
        seg_local = seg[idx] - c * CELLS_PER_CORE
        order = np.argsort(seg_local, kind="stable")
        seg_sorted = seg_local[order]
        cells, starts, counts = np.unique(
            seg_sorted, return_index=True, return_counts=True
        )
        o2 = np.argsort(-counts, kind="stable")
        cells, starts, counts = cells[o2], starts[o2], counts[o2]
        pts = idx[order]
        per_core.append((cells, starts, counts, pts))

    n_occ_max = max(len(pc[0]) for pc in per_core)
    G = -(-n_occ_max // CHUNK)
    if G % 2:
        G += 1  # fin works on chunk pairs
    kmax = int(max(pc[2].max() for pc in per_core))

    widths = np.zeros((kmax, G), np.int64)
    for cells, starts, counts, pts in per_core:
        n = len(cells)
        for g in range(G):
            lo, hi = g * CHUNK, min((g + 1) * CHUNK, n)
            if lo >= hi:
                continue
            cg = counts[lo:hi]
            for s in range(int(cg[0])):
                widths[s, g] = max(widths[s, g], int((cg > s).sum()))
    widths = np.minimum(-(-widths // 256) * 256, CHUNK)

    # items in s-major order; greedy-pack h1 blocks (w//2 cols) into
    # 2048-col groups so every item's h1 is contiguous in one tile
    items = []  # (s, g, w, grp, off)
    grp, off = 0, 0
    for s in range(kmax):
        for g in range(G):
            w = int(widths[s, g])
            if w == 0:
                continue
            h = w // 2
            if off + h > 2048:
                grp += 1
                off = 0
            items.append((s, g, w, grp, off))
            off += h
    G1 = grp + 1

    vox_all = np.zeros((N_CORES, 8, G1 * 2048), np.float32)
    rows_all = np.full((N_CORES, G * CHUNK), PAD, np.int64)

    for core, (cells, starts, counts, pts) in enumerate(per_core):
        n = len(cells)
        rows_all[core, :n] = cells
        for (s, g, w, gp, of) in items:
            lo = g * CHUNK
            ncell = max(0, min(w, n - lo))
            if ncell == 0:
                continue
            cnt = counts[lo : lo + ncell]
            so = np.minimum(s, cnt - 1)
            p_idx = pts[starts[lo : lo + ncell] + so]
            feats = np.zeros((w, 4), np.float32)
            feats[:ncell] = voxels[p_idx]
            h = w // 2
            packed = np.concatenate([feats[:h].T, feats[h:].T], axis=0)
            pc0 = gp * 2048 + of
            vox_all[core, :, pc0 : pc0 + h] = packed
    return (G, kmax, tuple(widths.flatten().tolist())), items, G1, \
        vox_all, rows_all


def _fold_weights(w1, g1, b1, m1, v1, w2, g2, b2, m2, v2,
                  w3, g3, b3, m3, v3, wc, bc):
    """Fold BN (eval) into the linear weights, build device layouts."""
    import ml_dtypes

    def sb(g, b, m, v):
        s = g / np.sqrt(v + EPS)
        return s.astype(np.float32), (b - m * s).astype(np.float32)

    s1, t1 = sb(g1, b1, m1, v1)
    s2, t2 = sb(g2, b2, m2, v2)
    s3, t3 = sb(g3, b3, m3, v3)
    w1f = (w1 * s1).astype(np.float32)  # [4,64]
    w2f = (w2 * s2).astype(np.float32)  # [64,128]
    w3f = (w3 * s3).astype(np.float32)  # [128,256]

    bf = ml_dtypes.bfloat16
    w1d8 = np.zeros((8, 128), np.float32)
    w1d8[0:4, 0:64] = w1f
    w1d8[4:8, 64:128] = w1f
    w2e = np.zeros((128, 128), np.float32)
    w2e[0:64] = w2f
    w2o = np.zeros((128, 128), np.float32)
    w2o[64:128] = w2f
    out = {
        "w1d8": w1d8.astype(bf),
        "w2e": w2e.astype(bf),
        "w2o": w2o.astype(bf),
        "w3a": np.ascontiguousarray(w3f[:, 0:128]).astype(bf),
        "w3b": np.ascontiguousarray(w3f[:, 128:256]).astype(bf),
        "wc0": np.ascontiguousarray(wc[0:128]).astype(np.float32).astype(bf),
        "wc1": np.ascontiguousarray(wc[128:256]).astype(np.float32).astype(bf),
        "t1d2": np.concatenate([t1, t1])[:, None].astype(np.float32),
        "t2": t2[:, None].astype(np.float32),
        "t3a": t3[0:128, None].astype(np.float32),
        "t3b": t3[128:256, None].astype(np.float32),
        "bc2": np.concatenate([bc, bc])[:, None].astype(np.float32),
    }
    return out


# ------------------------------------------------------------- bass program


def build_program(G, items, G1):
    nc = bacc.Bacc("TRN2", target_bir_lowering=False, debug=False,
                   num_devices=N_CORES)

    vox = nc.dram_tensor("vox", [8, G1 * 2048], BF16,
                         kind="ExternalInput").ap()
    wdram = {}
    for name, shape, dt in [
        ("w1d8", [8, 128], BF16), ("w2e", [128, 128], BF16),
        ("w2o", [128, 128], BF16),
        ("w3a", [128, 128], BF16), ("w3b", [128, 128], BF16),
        ("wc0", [128, 64], BF16), ("wc1", [128, 64], BF16),
        ("t1d2", [128, 1], F32), ("t2", [128, 1], F32),
        ("t3a", [128, 1], F32), ("t3b", [128, 1], F32),
        ("bc2", [128, 1], F32),
    ]:
        wdram[name] = (nc.dram_tensor(name, shape, dt,
                                      kind="ExternalInput").ap(), shape, dt)
    comp = nc.dram_tensor("comp", [64, G * CHUNK], BF16,
                          kind="ExternalOutput").ap()

    from contextlib import ExitStack
    with tile.TileContext(nc) as tc, ExitStack() as ctx:
        cpool = ctx.enter_context(tc.tile_pool(name="const", bufs=1))

        # weight/bias loads: spread over queues; w1-chain first so the
        # first p1 unit can start immediately
        _q = [nc.scalar, nc.gpsimd, nc.sync]
        wt = {}
        for i, name in enumerate(["w1d8", "t1d2", "w2e", "w2o", "t2",
                                  "w3a", "w3b", "t3a", "t3b",
                                  "wc0", "wc1", "bc2"]):
            ap, shape, dt = wdram[name]
            t = cpool.tile(shape, dt, tag=name)
            _q[i % 3].dma_start(out=t[:], in_=ap)
            wt[name] = t

        vox_sb = cpool.tile([8, G1 * 2048], BF16)
        for i in range(3):
            lo = (G1 * 2048 // 3) * i
            hi = (G1 * 2048 // 3) * (i + 1) if i < 2 else G1 * 2048
            _q[i].dma_start(out=vox_sb[:, lo:hi], in_=vox[:, lo:hi])

        accA = cpool.tile([128, G * CHUNK], BF16)
        accB = cpool.tile([128, G * CHUNK], BF16)
        w0 = {}
        for (s, g, w, gp, of) in items:
            w0[g] = max(w0.get(g, 0), w)
        for g in range(G):
            wg = w0.get(g, 0)
            if wg < CHUNK:
                nc.vector.memset(accA[:, g * CHUNK + wg : (g + 1) * CHUNK], 0.0)
                nc.vector.memset(accB[:, g * CHUNK + wg : (g + 1) * CHUNK], 0.0)

        h1p = ctx.enter_context(tc.tile_pool(name="h1p", bufs=4))
        h2p = ctx.enter_context(tc.tile_pool(name="h2p", bufs=5))
        tmp = ctx.enter_context(tc.tile_pool(name="tmp", bufs=4))
        scp = ctx.enter_context(tc.tile_pool(name="scp", bufs=2))
        psum = ctx.enter_context(tc.tile_pool(name="psum", bufs=2,
                                              space="PSUM"))

        # drain routing: debt-balanced over ACT / DVE (constants fit to
        # HW-measured slice durations at N=2048)
        debt = {"act": 0.0, "dve": 0.0}
        COST = {
            "act": lambda n: (n + 352.0) / 1.46,
            "dve": lambda n: (n + 120.0) / 1.13,
        }

        def br_auto(out_ap, in_ap, bias_ap):
            n = in_ap.shape[-1]
            eng = min(("act", "dve"), key=lambda e: debt[e] + COST[e](n))
            debt[eng] += COST[eng](n)
            if eng == "act":
                nc.scalar.activation(out_ap, in_ap, Relu, bias=bias_ap,
                                     scale=1.0)
            else:
                nc.vector.tensor_scalar(out_ap, in_ap, bias_ap, 0.0,
                                        op0=mybir.AluOpType.add,
                                        op1=mybir.AluOpType.max)

        h1t = {}

        def unit_p1(t):
            p = psum.tile([128, 2048], F32, tag="ps", space="PSUM",
                          name=f"p1_{t}")
            slab = vox_sb[:, t * 2048 : (t + 1) * 2048]
            for i in range(4):
                nc.tensor.matmul(p[:, i * 512 : (i + 1) * 512],
                                 wt["w1d8"][:],
                                 slab[:, i * 512 : (i + 1) * 512],
                                 start=True, stop=True)
            h1 = h1p.tile([128, 2048], BF16, tag="h1", name=f"h1_{t}")
            br_auto(h1[:], p[:], wt["t1d2"][:])
            h1t[t] = h1

        def seg(c0, width):
            # split [c0, c0+width) at 512-col PSUM bank boundaries
            j = c0
            while j < c0 + width:
                j1 = min((j // 512 + 1) * 512, c0 + width)
                yield j, j1
                j = j1

        def unit_p2(it):
            (s, g, w, gp, of) = it
            h = w // 2
            h1s = h1t[gp]
            p = psum.tile([128, 2048], F32, tag="ps", space="PSUM",
                          name=f"p2_{s}_{g}")[:, 0:w]
            for j, j1 in seg(0, h):
                nc.tensor.matmul(p[:, j:j1], wt["w2e"][:],
                                 h1s[:, of + j : of + j1],
                                 start=True, stop=True)
            for j, j1 in seg(h, h):
                nc.tensor.matmul(p[:, j:j1], wt["w2o"][:],
                                 h1s[:, of + j - h : of + j1 - h],
                                 start=True, stop=True)
            h2 = h2p.tile([128, 2048], BF16, tag="h2",
                          name=f"h2_{s}_{g}")[:, 0:w]
            br_auto(h2, p, wt["t2"][:])
            return h2

        def unit_p3(it, h2_ap, half):
            (s, g, w, gp, of) = it
            wst = wt["w3a"] if half == 0 else wt["w3b"]
            bias = wt["t3a"] if half == 0 else wt["t3b"]
            acc = (accA if half == 0 else accB)[:, g * CHUNK : g * CHUNK + w]
            p = psum.tile([128, 2048], F32, tag="ps", space="PSUM",
                          name=f"p3{half}_{s}_{g}")[:, 0:w]
            for j, j1 in seg(0, w):
                nc.tensor.matmul(p[:, j:j1], wst[:], h2_ap[:, j:j1],
                                 start=True, stop=True)
            if s == 0:
                br_auto(acc, p, bias[:])
            else:
                # relu-drain to temp (ACT/DVE), then 2x-rate bf16 SBUF
                # max into acc on DVE (acc >= 0 so relu commutes w/ max)
                tt = tmp.tile([128, 2048], BF16, tag="tt",
                              name=f"tt{half}_{s}_{g}")[:, 0:w]
                br_auto(tt, p, bias[:])
                debt["dve"] += (w / 2.0 + 151.0) / 0.96
                nc.vector.tensor_max(acc, tt, acc)

        def unit_fin(g):
            p = psum.tile([128, 2048], F32, tag="ps", space="PSUM",
                          name=f"pc_{g}")
            for q in range(2):
                cols = (g + q) * CHUNK
                dst = p[64 * q : 64 * q + 64, :]
                for j in range(0, 2048, 512):
                    nc.tensor.matmul(dst[:, j : j + 512], wt["wc0"][:],
                                     accA[:, cols + j : cols + j + 512],
                                     start=True, stop=False)
                    nc.tensor.matmul(dst[:, j : j + 512], wt["wc1"][:],
                                     accB[:, cols + j : cols + j + 512],
                                     start=False, stop=True)
            sc = scp.tile([128, 2048], BF16, tag="sc", name=f"sc_{g}")
            br_auto(sc[:], p[:], wt["bc2"][:])
            o = g * CHUNK
            nc.gpsimd.dma_start(out=comp[:, o : o + CHUNK], in_=sc[0:64, :])
            nc.gpsimd.dma_start(out=comp[:, o + CHUNK : o + 2 * CHUNK],
                                in_=sc[64:128, :])

        # ---- software-pipelined emission ----
        last_s = {}
        for (s, g, w, gp, of) in items:
            last_s[g] = s
        done = [False] * G
        finned = [False] * G

        def try_fins(g):
            done[g] = True
            gp = g - (g % 2)
            if done[gp] and done[gp + 1] and not finned[gp]:
                finned[gp] = True
                unit_fin(gp)

        have_items = {g for (_, g, _, _, _) in items}
        for g in range(G):
            if g not in have_items:
                done[g] = True
        for g in range(0, G, 2):
            if done[g] and done[g + 1]:
                finned[g] = True

        p2q = list(items)
        p3q = []

        def pump(grp_done_upto):
            while p3q:
                it, h2a = p3q.pop(0)
                unit_p3(it, h2a, 0)
                unit_p3(it, h2a, 1)
                if it[0] == last_s[it[1]]:
                    try_fins(it[1])
            while p2q and p2q[0][3] < grp_done_upto:
                it = p2q.pop(0)
                p3q.append((it, unit_p2(it)))

        for t in range(G1):
            unit_p1(t)
            pump(t)
        pump(G1)
        pump(G1)

    nc.compile()
    return nc


# ------------------------------------------------------------------ driver

_CACHE = {}


def kernel(voxels, coors, batch_size, w1, g1, b1, m1, v1,
           w2, g2, b2, m2, v2, w3, g3, b3, m3, v3, wc, bc,
           _trace=False):
    voxels = np.asarray(voxels, np.float32)
    coors = np.asarray(coors, np.int32)
    plan_key, items, G1, vox_all, rows_all = _build_plan_and_data(
        voxels, coors)
    G = plan_key[0]

    if plan_key not in _CACHE:
        _CACHE[plan_key] = build_program(G, items, G1)
    nc = _CACHE[plan_key]

    folded = _fold_weights(
        np.asarray(w1, np.float32), np.asarray(g1, np.float32),
        np.asarray(b1, np.float32), np.asarray(m1, np.float32),
        np.asarray(v1, np.float32),
        np.asarray(w2, np.float32), np.asarray(g2, np.float32),
        np.asarray(b2, np.float32), np.asarray(m2, np.float32),
        np.asarray(v2, np.float32),
        np.asarray(w3, np.float32), np.asarray(g3, np.float32),
        np.asarray(b3, np.float32), np.asarray(m3, np.float32),
        np.asarray(v3, np.float32),
        np.asarray(wc, np.float32), np.asarray(bc, np.float32))

    import ml_dtypes
    in_maps = [
        {"vox": vox_all[c].astype(ml_dtypes.bfloat16), **folded}
        for c in range(N_CORES)
    ]
    res = bass_utils.run_bass_kernel_spmd(
        nc, in_maps, core_ids=list(range(N_CORES)), trace=_trace)

    out = np.zeros((B, 64, GX * GY), np.float32)
    for c in range(N_CORES):
        cm = np.asarray(res.results[c]["comp"]).astype(np.float32)
        rows = rows_all[c]
        real = rows != PAD
        gcell = rows[real] + c * CELLS_PER_CORE
        b_core = c // (N_CORES // B)
        xy = gcell - b_core * (GX * GY)
        out[b_core][:, xy] = cm[:, real]
    out = out.reshape(B, 64, GX, GY)
    if _trace:
        return out, res
    return out
